# revision 17
# baseline (speedup 1.0000x reference)
"""3-layer GAT on 8 Trainium2 NeuronCores (Bass/Tile, SPMD).

Strategy (dst-sharded edge parallelism, shard-computed pack tables):
- Pad N to NPAD = 8*128*k nodes; core c owns the contiguous node range
  [c*NPC, (c+1)*NPC) and processes exactly the edges whose dst falls in
  its range (edges sorted by dst on host). Weights replicated.
- Host->device traffic is minimized (the axon tunnel runs at ~40 MB/s,
  so it dominates wall time): each core receives ONLY its own slice of
  x (transposed, 6.4 MB) plus edge index maps; the per-node feature
  table pack = h @ [W | W@al | W@ar] is computed for OWN nodes only and
  AllGathered on-device (NeuronLink), never shipped from host.
- Per layer: mm phase computes pack rows [feat | el | er] for own
  nodes; AllGather -> full table; edge phase: for each 128-edge tile,
  pack[src] rows arrive via indirect DMA gather and er[dst] via a
  second narrow indirect gather; one-hot matrices built on-device from
  dst%128 values (iota + is_equal) turn segment-sum into PE matmuls
  accumulated in PSUM per 128-node block. Softmax denominators are
  divided once per node post-aggregation (exp(e)/sum exp(e) == softmax
  exactly; no max-subtraction needed at these magnitudes).
- Edge-phase element-wise work is batched K tiles at a time with
  broadcast access patterns to cut instruction count.
- The built Bass program and the XLA/NEFF executable are cached across
  kernel() calls (jax persistent compilation cache), so warm calls skip
  the multi-second walrus compile.
"""
import os
import numpy as np
from contextlib import ExitStack

import jax

try:
    jax.config.update("jax_compilation_cache_dir",
                      os.path.expanduser("~/.cache/jax_comp_cache"))
    jax.config.update("jax_persistent_cache_min_compile_time_secs", 0.5)
    jax.config.update("jax_persistent_cache_min_entry_size_bytes", 0)
except Exception:
    pass

import concourse.bass as bass
import concourse.mybir as mybir
import concourse.tile as tile
from concourse.bass_utils import run_bass_kernel_spmd
from concourse.masks import make_identity

try:
    import bass_rust
except ImportError:  # pragma: no cover
    bass_rust = None

F32 = mybir.dt.float32
F16 = mybir.dt.float16
I32 = mybir.dt.int32
U16 = mybir.dt.uint16
I16 = mybir.dt.int16
BF16 = mybir.dt.bfloat16
I8 = mybir.dt.int8
U8 = mybir.dt.uint8
ALU = mybir.AluOpType
ACT = mybir.ActivationFunctionType
P = 128
NC = 8
NEG_SLOPE = 0.2
K_GRP = 9

_ws_ctr = [0]


def _split_waits(nc, limit=1):
    """This container's walrus encodes at most `limit` sem waits per
    instruction. Hoist extras onto same-engine NoOps placed before."""
    for fn in nc.m.functions:
        for bb in fn.blocks:
            insts = bb.instructions
            if not any(
                i.sync_info is not None and len(i.sync_info.on_wait) > limit
                for i in insts
            ):
                continue
            out = []
            for ins in insts:
                si = ins.sync_info
                if si is not None and len(si.on_wait) > limit:
                    waits = list(si.on_wait)
                    extra, keep = waits[:-limit], waits[-limit:]
                    for w in extra:
                        _ws_ctr[0] += 1
                        nop = mybir.InstNoOp(
                            name=f"I-waitsplit-{_ws_ctr[0]}", ins=[], outs=[]
                        )
                        nop.engine = ins.engine
                        nop.sync_info = bass_rust.SyncInfo(on_wait=[w], on_update=[])
                        out.append(nop)
                    ins.sync_info = bass_rust.SyncInfo(
                        on_wait=keep, on_update=list(si.on_update)
                    )
                out.append(ins)
            bb.instructions = out


def _pack_cols(n):  # pad row length to a 64-float multiple for clean strides
    return ((n + 63) // 64) * 64


def _layout(NPC, NT):
    """Section offsets (in u16 elements) of the consolidated input tensor.
    One buffer per core instead of ten: the axon tunnel charges ~90 ms of
    fixed overhead per device_put buffer, which dominated the wall time."""
    off, cur = {}, 0

    def add(name, n):
        nonlocal cur
        off[name] = cur
        cur = ((cur + n + 63) // 64) * 64

    add("x", 128 * NPC)          # i8 bits (2 per u16), view [256, NPC]
    add("xs", NPC)               # f16 per-node dequant scales, view [P, NB]
    add("idx", P * NT)           # u16
    add("dgi", P * NT // 2)      # u8 dst%128 (2 per u16), pad=255
    add("w0a", P * 264)          # bf16 bits
    add("w0b", P * 264)
    add("w1a", P * 264)
    add("w1b", P * 264)
    add("w2a", P * 66)
    add("w2b", P * 66)
    add("rwa", P * 64)
    add("rwb", P * 64)
    add("bias", 576)             # bf16 bits, one [1, 576] row
    off["_total"] = cur
    return off


def _build_program(NPAD, T_B, NB):
    NPC = NPAD // NC
    NT = NB * T_B

    nc = bass.Bass(num_devices=NC)

    F_ = (256, 256, 64)             # feature width per layer
    H_ = (4, 4, 1)                  # heads per layer
    C_ = tuple(f + 2 * h for f, h in zip(F_, H_))   # feat + el + er
    E_ = tuple(_pack_cols(c) for c in C_)           # padded pack row

    # ---- DRAM tensors ----
    lay = _layout(NPC, NT)
    mega = nc.dram_tensor("mega", [lay["_total"]], U16, kind="ExternalInput")

    def sec(name, n, dt=None):
        ap = mega[lay[name]:lay[name] + n]
        return ap.bitcast(dt) if dt is not None else ap

    xTo = sec("x", 128 * NPC).bitcast(I8).rearrange("(r c) -> r c", r=256)
    xs_src = sec("xs", NPC, F16).rearrange("(p c) -> p c", p=P)
    idx_src = sec("idx", P * NT).rearrange("(p c) -> p c", p=P)
    dgi_src = sec("dgi", P * NT // 2).bitcast(U8).rearrange("(p c) -> p c", p=P)
    wag_src = [
        (sec(f"w{i}a", P * c, BF16).rearrange("(p c) -> p c", p=P),
         sec(f"w{i}b", P * c, BF16).rearrange("(p c) -> p c", p=P))
        for i, c in enumerate(C_)
    ]
    rw2_src = (sec("rwa", P * 64, BF16).rearrange("(p c) -> p c", p=P),
               sec("rwb", P * 64, BF16).rearrange("(p c) -> p c", p=P))
    bia_src = sec("bias", 576, BF16).rearrange("(p c) -> p c", p=1)
    out2 = nc.dram_tensor("out2", [NPC, 64], F16, kind="ExternalOutput")

    pko = [
        nc.dram_tensor(f"pko{i}", [NPC + 1, e], F32) for i, e in enumerate(E_)
    ]
    pka = [
        nc.dram_tensor(f"pka{i}", [NPAD, e], F32, addr_space="Shared")
        for i, e in enumerate(E_)
    ]
    hown = nc.dram_tensor("hown", [NPC, 256], F32)
    agin = [nc.dram_tensor(f"agin{i}", [256, NPC], F32) for i in range(2)]

    with tile.TileContext(nc) as tc, ExitStack() as ctx:
        cst = ctx.enter_context(tc.tile_pool(name="cst", bufs=1))
        ld = ctx.enter_context(tc.tile_pool(name="ld", bufs=4))
        stg = ctx.enter_context(tc.tile_pool(name="stg", bufs=4))
        gp = ctx.enter_context(tc.tile_pool(name="gp", bufs=3))
        ep = ctx.enter_context(tc.tile_pool(name="ep", bufs=3))
        sm = ctx.enter_context(tc.tile_pool(name="sm", bufs=4))
        # PSUM: 8 banks/partition total; every tile is padded to one bank.
        mp = ctx.enter_context(tc.tile_pool(name="mp", bufs=2, space="PSUM"))
        m1p = ctx.enter_context(tc.tile_pool(name="m1p", bufs=2, space="PSUM"))
        rp_ = ctx.enter_context(tc.tile_pool(name="rp", bufs=1, space="PSUM"))
        agp = ctx.enter_context(tc.tile_pool(name="agp", bufs=2, space="PSUM"))

        # ---- constants ----
        idx_raw = ld.tile([P, NT], U16, tag="idxr")
        nc.sync.dma_start(idx_raw[:], idx_src)
        idx_sb = cst.tile([P, NT], I32, tag="idx")
        nc.vector.tensor_copy(idx_sb[:], idx_raw[:])
        dgi_raw = ld.tile([P, NT], U8, tag="dgir")
        nc.sync.dma_start(dgi_raw[:], dgi_src)
        dgi_sb = cst.tile([P, NT], I32, tag="dgi")
        nc.vector.tensor_copy(dgi_sb[:], dgi_raw[:])
        for b in range(NB):  # dst%128 -> dst%NPC (block-local maps are per-b)
            nc.vector.tensor_scalar_add(
                dgi_sb[:, b * T_B:(b + 1) * T_B], dgi_sb[:, b * T_B:(b + 1) * T_B],
                b * P)
        nc.vector.tensor_scalar_min(dgi_sb[:], dgi_sb[:], NPC)  # pads -> zero row
        dgi_f = cst.tile([P, NT], F32, tag="dgif")
        nc.vector.tensor_copy(dgi_f[:], dgi_sb[:])
        zro = cst.tile([P, max(E_)], F32, tag="zro")
        nc.vector.tensor_scalar_mul(zro[:], dgi_f[:, 0:max(E_)], 0.0)
        iota_i = cst.tile([P, P], I32, tag="ioi")
        nc.gpsimd.iota(iota_i[:], [[1, P]], channel_multiplier=0)
        iota_f = cst.tile([P, P], F32, tag="iof")
        nc.vector.tensor_copy(iota_f[:], iota_i[:])
        ident = cst.tile([P, P], F32, tag="id")
        make_identity(nc, ident[:])
        wag_sb = []
        for i, c in enumerate(C_):
            tr = ld.tile([P, 2, c], BF16, tag="wagr", name="wagr")
            nc.sync.dma_start(tr[:, 0, :], wag_src[i][0])
            nc.sync.dma_start(tr[:, 1, :], wag_src[i][1])
            t = cst.tile([P, 2, c], F32, tag=f"wag{i}")
            nc.vector.tensor_copy(t[:], tr[:])
            wag_sb.append(t)
        rw2r = ld.tile([P, 2, 64], BF16, tag="rw2r")
        nc.sync.dma_start(rw2r[:, 0, :], rw2_src[0])
        nc.sync.dma_start(rw2r[:, 1, :], rw2_src[1])
        rw2_sb = cst.tile([P, 2, 64], F32, tag="rw2")
        nc.vector.tensor_copy(rw2_sb[:], rw2r[:])
        xs_raw = ld.tile([P, NB], F16, tag="xsr")
        nc.sync.dma_start(xs_raw[:], xs_src)
        xs_sb = cst.tile([P, NB], F32, tag="xs")
        nc.vector.tensor_copy(xs_sb[:], xs_raw[:])
        bia_raw = ld.tile([1, 576], BF16, tag="biar")
        nc.sync.dma_start(bia_raw[:], bia_src)
        bia_row = ld.tile([1, 576], F32, tag="biarf")
        nc.vector.tensor_copy(bia_row[:], bia_raw[:])
        ones_r = ld.tile([1, P], F32, tag="ones")
        nc.vector.tensor_scalar_mul(ones_r[:], iota_f[0:1, :], 0.0)
        nc.vector.tensor_scalar_add(ones_r[:], ones_r[:], 1.0)
        bia_all = cst.tile([P, 576], F32, tag="biaall")
        for c0_, c1_ in ((0, 256), (256, 512), (512, 576)):
            bp = mp.tile([P, max(C_)], F32, tag="mmps", name="bp",
                         space="PSUM")[:, 0:c1_ - c0_]
            nc.tensor.matmul(out=bp[:], lhsT=ones_r[:],
                             rhs=bia_row[:, c0_:c1_], start=True, stop=True)
            nc.vector.tensor_copy(bia_all[:, c0_:c1_], bp[:])
        bia_sb = [bia_all[:, 0:256], bia_all[:, 256:512], bia_all[:, 512:576]]

        for i, e in enumerate(E_):
            nc.sync.dma_start(pko[i][NPC:NPC + 1, :], zro[0:1, 0:e])

        def mm_phase(L, h_src):
            """pack rows = h @ [W | W@al | W@ar] for OWN nodes only."""
            C, E = C_[L], E_[L]
            wt = wag_sb[L]
            for b in range(NB):
                c0 = ld.tile([P, P], F32, tag="c0")
                c1 = ld.tile([P, P], F32, tag="c1")
                if L == 0:  # x ships as int8 (scale folded into wag0)
                    c0h = ld.tile([P, P], I8, tag="c0h")
                    c1h = ld.tile([P, P], I8, tag="c1h")
                    nc.sync.dma_start(c0h[:], h_src[0:P, b * P:(b + 1) * P])
                    nc.sync.dma_start(c1h[:], h_src[P:2 * P, b * P:(b + 1) * P])
                    nc.vector.tensor_copy(c0[:], c0h[:])
                    nc.vector.tensor_copy(c1[:], c1h[:])
                else:
                    nc.sync.dma_start(c0[:], h_src[0:P, b * P:(b + 1) * P])
                    nc.sync.dma_start(c1[:], h_src[P:2 * P, b * P:(b + 1) * P])
                ps = mp.tile([P, max(C_)], F32, tag="mmps", name="mmps",
                             space="PSUM")[:, 0:C]
                nc.tensor.matmul(out=ps[:], lhsT=c0[:], rhs=wt[:, 0, :],
                                 start=True, stop=False)
                nc.tensor.matmul(out=ps[:], lhsT=c1[:], rhs=wt[:, 1, :],
                                 start=False, stop=True)
                st = stg.tile([P, max(E_)], F32, tag="stg")
                if L == 0:  # fold per-node int8 dequant scale into the copy
                    nc.vector.tensor_tensor(
                        out=st[:, 0:C], in0=ps[:],
                        in1=xs_sb[:, b:b + 1].to_broadcast([P, C]), op=ALU.mult)
                else:
                    nc.vector.tensor_copy(st[:, 0:C], ps[:])
                nc.sync.dma_start(pko[L][b * P:(b + 1) * P, :], st[:, 0:E])

        def edge_phase(L, write_sinks):
            F, H, E = F_[L], H_[L], E_[L]
            pk = pka[L]
            for b in range(NB):
                agf = agp.tile([P, 272], F32, tag="agg", name="agg", space="PSUM")
                agg = agf[:, 0:F]
                den = agf[:, F:F + H]
                for t0 in range(0, T_B, K_GRP):
                    k = min(K_GRP, T_B - t0)
                    tt0 = b * T_B + t0
                    gw = gp.tile([P, K_GRP, E], F32, tag="gw")
                    ge = gp.tile([P, K_GRP, H], F32, tag="ge")
                    m1w = ep.tile([P, K_GRP, P], F32, tag="m1w")
                    for j in range(k):
                        nc.gpsimd.indirect_dma_start(
                            out=gw[:, j, :], out_offset=None, in_=pk[:],
                            in_offset=bass.IndirectOffsetOnAxis(
                                ap=idx_sb[:, tt0 + j:tt0 + j + 1], axis=0))
                        nc.gpsimd.indirect_dma_start(
                            out=ge[:, j, :], out_offset=None, in_=pko[L][:],
                            element_offset=F + H,
                            in_offset=bass.IndirectOffsetOnAxis(
                                ap=dgi_sb[:, tt0 + j:tt0 + j + 1], axis=0))
                    # one-hot of (dst_local - b*128) for k tiles in one op;
                    # pad slots hold dgi=NPC which never lands in [0,128)
                    nc.vector.scalar_tensor_tensor(
                        out=m1w[:, 0:k, :],
                        in0=dgi_f[:, tt0:tt0 + k].unsqueeze(2)
                            .to_broadcast([P, k, P]),
                        scalar=float(-b * P),
                        in1=iota_f[:].unsqueeze(1).to_broadcast([P, k, P]),
                        op0=ALU.add, op1=ALU.is_equal)
                    ew = ep.tile([P, K_GRP, H], F32, tag="ew")
                    nc.vector.tensor_add(ew[:, 0:k, :], gw[:, 0:k, F:F + H],
                                         ge[:, 0:k, :])
                    nc.vector.scalar_tensor_tensor(
                        out=ew[:, 0:k, :], in0=ew[:, 0:k, :], scalar=NEG_SLOPE,
                        in1=ew[:, 0:k, :], op0=ALU.mult, op1=ALU.max)
                    sc = ep.tile([P, K_GRP, F + H], F32, tag="sc")
                    nc.scalar.activation(sc[:, 0:k, F:F + H], ew[:, 0:k, :],
                                         ACT.Exp)
                    nc.vector.tensor_tensor(
                        out=sc[:, 0:k, 0:F].rearrange("p k (h d) -> p k h d", h=H),
                        in0=gw[:, 0:k, 0:F].rearrange("p k (h d) -> p k h d", h=H),
                        in1=sc[:, 0:k, F:F + H].unsqueeze(3)
                            .to_broadcast([P, k, H, F // H]),
                        op=ALU.mult)
                    for j in range(k):
                        nc.tensor.matmul(out=agf[:, 0:F + H], lhsT=m1w[:, j, :],
                                         rhs=sc[:, j, :],
                                         start=(t0 + j == 0),
                                         stop=(t0 + j == T_B - 1))
                # epilogue
                den_c = ep.tile([P, H], F32, tag="denc")
                nc.vector.tensor_scalar_max(den_c[:], den[:], 1e-30)
                rec = ep.tile([P, H], F32, tag="rec")
                nc.vector.reciprocal(rec[:], den_c[:])
                o = ep.tile([P, F], F32, tag="o")
                nc.vector.tensor_tensor(
                    out=o[:].rearrange("p (h d) -> p h d", h=H),
                    in0=agg[:].rearrange("p (h d) -> p h d", h=H),
                    in1=rec[:].to_broadcast([P, H, F // H]), op=ALU.mult)
                write_sinks(b, o)

        def sink_l0(b, o):
            nc.vector.tensor_add(o[:], o[:], bia_sb[0])
            _elu(o)
            nc.sync.dma_start(hown[b * P:(b + 1) * P, :], o[:])
            _write_agin(agin[0], b, o)

        def sink_l1(b, o):
            hb = ld.tile([P, 256], F32, tag="hb")
            nc.sync.dma_start(hb[:], hown[b * P:(b + 1) * P, :])
            nc.vector.tensor_add(o[:], o[:], hb[:])
            nc.vector.tensor_add(o[:], o[:], bia_sb[1])
            _elu(o)
            _write_agin(agin[1], b, o)

        def sink_l2(b, o):
            r0 = ld.tile([P, P], F32, tag="r0")
            r1 = ld.tile([P, P], F32, tag="r1")
            nc.sync.dma_start(r0[:], agin[1][0:P, b * P:(b + 1) * P])
            nc.sync.dma_start(r1[:], agin[1][P:2 * P, b * P:(b + 1) * P])
            rp = rp_.tile([P, 64], F32, tag="resps", space="PSUM")
            nc.tensor.matmul(out=rp[:], lhsT=r0[:], rhs=rw2_sb[:, 0, :],
                             start=True, stop=False)
            nc.tensor.matmul(out=rp[:], lhsT=r1[:], rhs=rw2_sb[:, 1, :],
                             start=False, stop=True)
            nc.vector.tensor_add(o[:], o[:], rp[:])
            nc.vector.tensor_add(o[:], o[:], bia_sb[2])
            of = sm.tile([P, 64], F16, tag="of")
            nc.vector.tensor_copy(of[:], o[:])
            nc.sync.dma_start(out2[b * P:(b + 1) * P, :], of[:])

        def _elu(o):
            mx = sm.tile([P, 256], F32, tag="mx")
            nc.vector.tensor_scalar_max(mx[:], o[:], 0.0)
            mn = sm.tile([P, 256], F32, tag="mn")
            nc.vector.tensor_scalar_min(mn[:], o[:], 0.0)
            exn = sm.tile([P, 256], F32, tag="exn")
            nc.scalar.activation(exn[:], mn[:], ACT.Exp)
            nc.vector.scalar_tensor_tensor(
                out=o[:], in0=exn[:], scalar=-1.0, in1=mx[:],
                op0=ALU.add, op1=ALU.add)

        def _write_agin(ag, b, o):
            t1 = m1p.tile([P, P], F32, tag="m1tps", space="PSUM")
            nc.tensor.transpose(out=t1[:], in_=o[:, 0:P], identity=ident[:])
            ot1 = sm.tile([P, P], F32, tag="ot1")
            nc.vector.tensor_copy(ot1[:], t1[:])
            nc.sync.dma_start(ag[0:P, b * P:(b + 1) * P], ot1[:])
            t2 = m1p.tile([P, P], F32, tag="m1tps", space="PSUM")
            nc.tensor.transpose(out=t2[:], in_=o[:, P:2 * P], identity=ident[:])
            ot2 = sm.tile([P, P], F32, tag="ot2")
            nc.vector.tensor_copy(ot2[:], t2[:])
            nc.sync.dma_start(ag[P:2 * P, b * P:(b + 1) * P], ot2[:])

        def allgather(L):
            tc.strict_bb_all_engine_barrier()
            nc.gpsimd.collective_compute(
                "AllGather", ALU.bypass, replica_groups=[list(range(NC))],
                ins=[pko[L][0:NPC, :]], outs=[pka[L][:]])
            tc.strict_bb_all_engine_barrier()

        # ---- layer 0 ----
        mm_phase(0, xTo)
        allgather(0)
        edge_phase(0, sink_l0)
        tc.strict_bb_all_engine_barrier()
        # ---- layer 1 ----
        mm_phase(1, agin[0])
        allgather(1)
        edge_phase(1, sink_l1)
        tc.strict_bb_all_engine_barrier()
        # ---- layer 2 ----
        mm_phase(2, agin[1])
        allgather(2)
        edge_phase(2, sink_l2)

    _split_waits(nc, limit=1)
    return nc


_PROG_CACHE = {}


def prepare(**inputs):
    x = np.asarray(inputs["x"], dtype=np.float32)
    src = np.asarray(inputs["src"], dtype=np.int64)
    dst = np.asarray(inputs["dst"], dtype=np.int64)
    N, IND = x.shape
    NPAD = ((N + NC * P - 1) // (NC * P)) * (NC * P)
    NPC = NPAD // NC
    NB = NPC // P

    # ---- host-side graph preprocessing (sharding) ----
    core = dst // NPC
    blk = (dst % NPC) // P
    order = np.lexsort((src, blk, core))
    src_s, dst_s, core_s, blk_s = (
        src[order], dst[order], core[order], blk[order])
    # per (core, block) counts
    counts = np.zeros((NC, NB), dtype=np.int64)
    np.add.at(counts, (core_s, blk_s), 1)
    T_B = int(np.max((counts + P - 1) // P))
    NT = NB * T_B
    idx_all = np.zeros((NC, NT * P), dtype=np.uint16)         # pad idx -> row 0
    dgi_all = np.full((NC, NT * P), 255, dtype=np.uint8)      # dst%128; pad 255
    for c in range(NC):
        m = core_s == c
        bc = np.concatenate([[0], np.cumsum(counts[c])])
        sc_, dc_ = src_s[m], dst_s[m]
        for b in range(NB):
            seg = slice(bc[b], bc[b + 1])
            n = bc[b + 1] - bc[b]
            base = b * T_B * P
            idx_all[c, base:base + n] = sc_[seg]
            dgi_all[c, base:base + n] = dc_[seg] % P
    # wrap position i -> (partition i%128, col i//128)
    idx_maps = idx_all.reshape(NC, NT, P).transpose(0, 2, 1)   # [NC, P, NT]
    dgi_maps = dgi_all.reshape(NC, NT, P).transpose(0, 2, 1)

    # ---- weights prep: wag = [W | W@al | W@ar] ----
    def aug(W, al, ar):
        H, D = al.shape
        alc = np.stack([W[:, h * D:(h + 1) * D] @ al[h] for h in range(H)], axis=1)
        arc = np.stack([W[:, h * D:(h + 1) * D] @ ar[h] for h in range(H)], axis=1)
        return np.concatenate([W, alc, arc], axis=1).astype(np.float32)

    wag0 = aug(inputs["W0"], inputs["al0"], inputs["ar0"])
    wag1 = aug(inputs["W1"], inputs["al1"], inputs["ar1"])
    wag2 = aug(inputs["W2"], inputs["al2"], inputs["ar2"])
    b0 = np.asarray(inputs["b0"], np.float32)
    b1 = np.asarray(inputs["b1"], np.float32)
    b2 = np.asarray(inputs["b2"], np.float32)
    rw2 = np.asarray(inputs["res_w2"], np.float32)

    xpad = np.zeros((NPAD, IND), np.float32)
    xpad[:N] = x
    xsc = np.abs(xpad).max(axis=1) / 127.0           # per-node scale
    xsc = np.maximum(xsc, 1e-12).astype(np.float32)
    xq = np.rint(xpad / xsc[:, None]).astype(np.int8)
    xsc16 = xsc.astype(np.float16)                    # shipped; device multiplies

    key = (NPAD, T_B, NB)
    if key not in _PROG_CACHE:
        prog = _build_program(NPAD, T_B, NB)
        raw = prog.to_json_bytes()      # BIR bytes are deterministic for a
        prog.to_json_bytes = lambda: raw  # built program; skip re-serializing
        _PROG_CACHE[key] = prog           # ~0.12 s on every later lowering
    nc = _PROG_CACHE[key]

    import ml_dtypes

    lay = _layout(NPC, NB * T_B)

    def u16(arr):
        return np.ascontiguousarray(arr).view(np.uint16).ravel()

    def chunks2(W):  # [256, C] -> [2, 128, C] in bf16
        return np.stack([W[0:P], W[P:2 * P]]).astype(ml_dtypes.bfloat16)

    # weight/bias sections are identical on every core; fill once
    base = np.zeros(lay["_total"], np.uint16)

    def put(m, name, arr):
        flat = u16(arr)
        m[lay[name]:lay[name] + flat.size] = flat

    for i, wg in enumerate((wag0, wag1, wag2)):
        ch = chunks2(wg)
        put(base, f"w{i}a", ch[0])
        put(base, f"w{i}b", ch[1])
    chr_ = chunks2(rw2)
    put(base, "rwa", chr_[0])
    put(base, "rwb", chr_[1])
    put(base, "bias",
        np.concatenate([b0, b1, b2])[None, :].astype(ml_dtypes.bfloat16))

    in_maps = []
    for c in range(NC):
        m = base.copy()
        put(m, "x", xq[c * NPC:(c + 1) * NPC].T)
        put(m, "xs", xsc16[c * NPC:(c + 1) * NPC].reshape(NB, P).T)
        put(m, "idx", idx_maps[c])
        put(m, "dgi", dgi_maps[c])
        in_maps.append({"mega": m})

    return nc, in_maps, N


def kernel(**inputs):
    nc, in_maps, N = prepare(**inputs)
    import time as _time
    global LAST_EXEC_WALL
    res = None
    for attempt in range(3):
        try:
            _t0 = _time.time()
            res = run_bass_kernel_spmd(nc, in_maps, list(range(NC)))
            LAST_EXEC_WALL = _time.time() - _t0
            break
        except Exception:
            # transient device wedge (NRT_EXEC_UNIT_UNRECOVERABLE) usually
            # clears on retry; re-raise only if it persists
            if attempt == 2:
                raise
    out = np.concatenate([res.results[c]["out2"] for c in range(NC)], axis=0)
    return out[:N].astype(np.float32)
